# revision 14
# baseline (speedup 1.0000x reference)
"""Trainium2 Bass kernel for nn_CTCModel (bidirectional GRU CTC model).

Sharding: direction-split data parallel. Cores 0-3 run the forward GRU for
8 sequences each; cores 4-7 run the backward GRU (on host-reversed input)
for the same sequences. Each core:
  phase 1: bf16 MLP (relu(x@w1+b1) -> relu(@w2+b2)) and precompute of the
           x-dependent GRU terms gx = h2@wg_x+bg, cx = h2@wc_x+bc, with a
           +1e9 bias folded into the u-gate for t >= seq_len (freezes the
           recurrent state exactly, since u=sigmoid(1e9)=1). Stored to a
           DRAM stream in f32, one block per U-step sub-chunk.
  scan:    1000 sequential GRU steps, h kept feature-major [128, 4*8].
           Per step the Pool engine preloads gx/cx into PSUM two steps
           ahead; the gate/cand matmuls accumulate onto them (start=False)
           so sigmoid/tanh read PSUM directly with no vector adds on the
           critical path. r-gate matmuls are emitted before u-gate ones so
           sigmoid(r) overlaps the u-gate matmuls. The state update is
           computed twice in parallel: Vector writes f32 hf, Pool writes
           bf16 into a per-sub-chunk history tile consumed by the (batched,
           per-sub-chunk) output projection h @ wf_half.
Host glue: shard/reverse/transpose inputs, block weights, and combine
partial logits: logits = where(t < len, pf + pb + bf, bf).
"""
import os
import sys

sys.path.insert(0, "/opt/trn_rl_repo")

import numpy as np
import ml_dtypes

import concourse.bass as bass
import concourse.tile as tile
from concourse import bacc, mybir
from concourse.bass import ts
from concourse.bass_utils import run_bass_kernel_spmd

AF = mybir.ActivationFunctionType
F32 = mybir.dt.float32
BF16 = mybir.dt.bfloat16

B, T_FULL, F, H, C = 32, 1000, 161, 512, 62
NCORES = 8
NSEQ = 8  # sequences per core
U_DEF = 20  # steps per sub-chunk in the scan loop
MASK_BIG = 1.0e9
KB = H // 128  # 4 k-blocks of the hidden dim
GW = 2 * KB * NSEQ  # 64: gate cols per step (8 m-tiles x 8 seqs)
CW = KB * NSEQ  # 32: cand cols per step
SC = GW + CW  # 96: stream cols per step

LAST_RESULT = None  # BassKernelResults of the most recent run (for test.py)

_NC_CACHE = {}


def build_nc(T=T_FULL, U=U_DEF, ncores=NCORES):
    """Build + compile the per-core Bass program (same program on all cores)."""
    key = (T, U, ncores)
    if key in _NC_CACHE:
        return _NC_CACHE[key]
    assert T % (2 * U) == 0
    NT = T // (2 * U)
    NSUB = T // U
    N = T * NSEQ  # flattened (t, b) rows
    CH_T = (64 // U) * U if U <= 64 else U  # phase-1 chunk t-width
    assert CH_T % U == 0

    nc = bacc.Bacc("TRN2", target_bir_lowering=False, debug=False,
                   num_devices=ncores)

    # ---- DRAM I/O ----
    xT_d = nc.dram_tensor("xT", [F, T, NSEQ], BF16, kind="ExternalInput").ap()
    w1_d = nc.dram_tensor("w1", [F, H], BF16, kind="ExternalInput").ap()
    b1_d = nc.dram_tensor("b1", [H], F32, kind="ExternalInput").ap()
    w2_d = nc.dram_tensor("w2", [H, H], BF16, kind="ExternalInput").ap()
    b2_d = nc.dram_tensor("b2", [H], F32, kind="ExternalInput").ap()
    wgx_d = nc.dram_tensor("wgx", [H, 2 * H], BF16, kind="ExternalInput").ap()
    bg_d = nc.dram_tensor("bg", [2 * H], F32, kind="ExternalInput").ap()
    wcx_d = nc.dram_tensor("wcx", [H, H], BF16, kind="ExternalInput").ap()
    bc_d = nc.dram_tensor("bc", [H], F32, kind="ExternalInput").ap()
    wgh_d = nc.dram_tensor("wgh", [128, 2 * KB * KB * 128], BF16,
                           kind="ExternalInput").ap()  # [128, 4096]
    wch_d = nc.dram_tensor("wch", [128, KB * KB * 128], BF16,
                           kind="ExternalInput").ap()  # [128, 2048]
    wf_d = nc.dram_tensor("wf", [128, KB * C], BF16, kind="ExternalInput").ap()
    id_d = nc.dram_tensor("ident", [128, 128], BF16, kind="ExternalInput").ap()
    mask_d = nc.dram_tensor("maskb", [128, N], F32, kind="ExternalInput").ap()
    logits_d = nc.dram_tensor("logits", [NSUB, C, U * NSEQ], F32,
                              kind="ExternalOutput").ap()

    with tile.TileContext(nc) as tc:
        with tc.tile_pool(name="dram", bufs=1, space="DRAM") as dpool:
            # stream blocks: one [128, U*SC] block per sub-chunk; 2 pad
            # blocks for the software-pipeline prefetch overrun at the end.
            stream = dpool.tile([NSUB + 2, 128, U * SC], BF16)

            # ================= phase 1 =================
            with tc.tile_pool(name="p1w", bufs=1) as p1w, \
                 tc.tile_pool(name="p1work", bufs=2) as p1wk, \
                 tc.tile_pool(name="p1ps", bufs=2, space="PSUM") as p1ps:
                w1a = p1w.tile([128, H], BF16)
                nc.sync.dma_start(w1a[:], w1_d[0:128, :])
                w1b = p1w.tile([F - 128, H], BF16)
                nc.sync.dma_start(w1b[:], w1_d[128:F, :])
                w2t = p1w.tile([128, KB, H], BF16)
                nc.sync.dma_start(w2t[:], w2_d.rearrange("(k p) m -> p k m", p=128))
                wgxt = p1w.tile([128, KB, 2 * H], BF16)
                nc.sync.dma_start(wgxt[:], wgx_d.rearrange("(k p) m -> p k m", p=128))
                wcxt = p1w.tile([128, KB, H], BF16)
                nc.sync.dma_start(wcxt[:], wcx_d.rearrange("(k p) m -> p k m", p=128))
                b1t = p1w.tile([128, KB], F32)
                nc.sync.dma_start(b1t[:], b1_d.rearrange("(m p) -> p m", p=128))
                b2t = p1w.tile([128, KB], F32)
                nc.sync.dma_start(b2t[:], b2_d.rearrange("(m p) -> p m", p=128))
                bgt = p1w.tile([128, 2 * KB], F32)
                nc.sync.dma_start(bgt[:], bg_d.rearrange("(m p) -> p m", p=128))
                bct = p1w.tile([128, KB], F32)
                nc.sync.dma_start(bct[:], bc_d.rearrange("(m p) -> p m", p=128))

                n_chunks = (T + CH_T - 1) // CH_T
                for ci in range(n_chunks):
                    t0 = ci * CH_T
                    tw = min(CH_T, T - t0)
                    ns = tw // U  # sub-chunks in this chunk
                    cw = tw * NSEQ
                    c0 = t0 * NSEQ
                    stripe = p1wk.tile([128, CH_T // U, U, SC], BF16,
                                       tag="stripe")

                    xa = p1wk.tile([128, 512], BF16, tag="xa")
                    nc.sync.dma_start(
                        xa[:, 0:cw],
                        xT_d[0:128, t0:t0 + tw, :].rearrange("f t b -> f (t b)"))
                    xb = p1wk.tile([F - 128, 512], BF16, tag="xb")
                    nc.sync.dma_start(
                        xb[:, 0:cw],
                        xT_d[128:F, t0:t0 + tw, :].rearrange("f t b -> f (t b)"))
                    maskt = p1wk.tile([128, 512], F32, tag="maskt")
                    nc.sync.dma_start(maskt[:, 0:cw], mask_d[:, c0:c0 + cw])

                    # MLP layer 1: h1 = relu(w1.T @ x + b1)
                    h1t = p1wk.tile([128, KB, 512], BF16, tag="h1t")
                    for m in range(KB):
                        ps = p1ps.tile([128, 512], F32, tag="ps1")
                        nc.tensor.matmul(ps[:, 0:cw], w1a[:, ts(m, 128)],
                                         xa[:, 0:cw], start=True, stop=False)
                        nc.tensor.matmul(ps[:, 0:cw], w1b[:, ts(m, 128)],
                                         xb[:, 0:cw], start=False, stop=True)
                        nc.scalar.activation(h1t[:, m, 0:cw], ps[:, 0:cw],
                                             AF.Relu, bias=b1t[:, m:m + 1])
                    # MLP layer 2
                    h2t = p1wk.tile([128, KB, 512], BF16, tag="h2t")
                    for m in range(KB):
                        ps = p1ps.tile([128, 512], F32, tag="ps2")
                        for k in range(KB):
                            nc.tensor.matmul(ps[:, 0:cw],
                                             w2t[:, k, ts(m, 128)],
                                             h1t[:, k, 0:cw],
                                             start=(k == 0), stop=(k == KB - 1))
                        nc.scalar.activation(h2t[:, m, 0:cw], ps[:, 0:cw],
                                             AF.Relu, bias=b2t[:, m:m + 1])
                    # gate precompute gxb = h2 @ wg_x + bg (+ mask on u half)
                    for m in range(2 * KB):
                        ps = p1ps.tile([128, 512], F32, tag="psg")
                        for k in range(KB):
                            nc.tensor.matmul(ps[:, 0:cw],
                                             wgxt[:, k, ts(m, 128)],
                                             h2t[:, k, 0:cw],
                                             start=(k == 0), stop=(k == KB - 1))
                        gdst = stripe[:, 0:ns, :, m * NSEQ:(m + 1) * NSEQ]
                        gsrc = ps[:, 0:cw].rearrange("p (s t b) -> p s t b",
                                                     t=U, b=NSEQ)
                        if m < KB:
                            nc.scalar.activation(gdst, gsrc, AF.Identity,
                                                 bias=bgt[:, m:m + 1])
                        else:
                            nc.vector.scalar_tensor_tensor(
                                gdst, gsrc, bgt[:, m:m + 1],
                                maskt[:, 0:cw].rearrange(
                                    "p (s t b) -> p s t b", t=U, b=NSEQ),
                                mybir.AluOpType.add, mybir.AluOpType.add)
                    # cand precompute cxb = h2 @ wc_x + bc
                    for m in range(KB):
                        ps = p1ps.tile([128, 512], F32, tag="psc")
                        for k in range(KB):
                            nc.tensor.matmul(ps[:, 0:cw],
                                             wcxt[:, k, ts(m, 128)],
                                             h2t[:, k, 0:cw],
                                             start=(k == 0), stop=(k == KB - 1))
                        cdst = stripe[:, 0:ns, :,
                                      GW + m * NSEQ:GW + (m + 1) * NSEQ]
                        csrc = ps[:, 0:cw].rearrange("p (s t b) -> p s t b",
                                                     t=U, b=NSEQ)
                        nc.scalar.activation(cdst, csrc, AF.Identity,
                                             bias=bct[:, m:m + 1])
                    # store assembled sub-chunk blocks (contiguous per
                    # partition: U*SC elements -> 128 descriptors)
                    for s in range(ns):
                        nc.sync.dma_start(
                            stream[t0 // U + s, :, :],
                            stripe[:, s, :, :].rearrange("p t c -> p (t c)"))

            # ================= scan =================
            with tc.tile_pool(name="scw", bufs=1) as scw, \
                 tc.tile_pool(name="scstep", bufs=2) as sbp, \
                 tc.tile_pool(name="lstp", bufs=2) as lstp, \
                 tc.tile_pool(name="psw", bufs=1, space="PSUM") as pswp, \
                 tc.tile_pool(name="psl", bufs=2, space="PSUM") as pslp:
                wghs = scw.tile([128, 2 * KB * KB * 128], BF16)
                nc.sync.dma_start(wghs[:], wgh_d[:])
                wchs = scw.tile([128, KB * KB * 128], BF16)
                nc.sync.dma_start(wchs[:], wch_d[:])
                wfs = scw.tile([128, KB * C], BF16)
                nc.sync.dma_start(wfs[:], wf_d[:])
                identw = scw.tile([128, 128], BF16)
                nc.sync.dma_start(identw[:], id_d[:])
                hf = scw.tile([128, CW], F32)
                nc.vector.memset(hf[:], 0.0)
                histA = scw.tile([128, U, CW], BF16)
                histB = scw.tile([128, U, CW], BF16)
                nc.gpsimd.memset(histB[:], 0.0)
                streamA = scw.tile([128, U * SC], BF16)
                streamB = scw.tile([128, U * SC], BF16)

                sviewB = stream[1:NSUB + 1].rearrange("(n s) f w -> n s f w",
                                                      s=2)
                sviewA2 = stream[2:NSUB + 2].rearrange("(n s) f w -> n s f w",
                                                       s=2)
                lview = logits_d.rearrange("(n s) c w -> n s c w", s=2)

                NS2 = 2 * U  # steps per loop body (sub-chunks A + B)
                # persistent even/odd PSUM tiles for the per-step preloads
                # (one full bank each; 6 + 2 logits banks = all 8)
                psrs = [pswp.tile([128, CW], F32, name=f"psr{i}")
                        for i in range(2)]
                psus = [pswp.tile([128, CW], F32, name=f"psu{i}")
                        for i in range(2)]
                pscs = [pswp.tile([128, CW], F32, name=f"psc{i}")
                        for i in range(2)]

                def emit_step(j, hb_prev, hist_dst):
                    """One GRU step. hb_prev: [128, CW] bf16 AP of h_{t-1};
                    hist_dst: [128, CW] bf16 AP to write h_t into."""
                    S = streamA if (j % NS2) < U else streamB
                    u = j % U
                    pr, pu, pc = psrs[j % 2], psus[j % 2], pscs[j % 2]
                    # each PSUM group is seeded with the x-dependent term via
                    # an identity matmul (start=True), then the recurrent
                    # matmuls accumulate onto it. r-gate matmuls come first
                    # so sigmoid(r) can start while the u-gate matmuls run.
                    nc.tensor.matmul(pr[:], identw[:],
                                     S[:, u * SC:u * SC + CW],
                                     start=True, stop=False,
                                     skip_group_check=True)
                    for m in range(KB):
                        for k in range(KB):
                            nc.tensor.matmul(
                                pr[:, m * NSEQ:(m + 1) * NSEQ],
                                wghs[:, ts(k * 2 * KB + m, 128)],
                                hb_prev[:, k * NSEQ:(k + 1) * NSEQ],
                                start=False,
                                stop=(m == KB - 1 and k == KB - 1),
                                skip_group_check=True)
                    nc.tensor.matmul(pu[:], identw[:],
                                     S[:, u * SC + CW:u * SC + GW],
                                     start=True, stop=False,
                                     skip_group_check=True)
                    for m in range(KB):
                        for k in range(KB):
                            nc.tensor.matmul(
                                pu[:, m * NSEQ:(m + 1) * NSEQ],
                                wghs[:, ts(k * 2 * KB + KB + m, 128)],
                                hb_prev[:, k * NSEQ:(k + 1) * NSEQ],
                                start=False,
                                stop=(m == KB - 1 and k == KB - 1),
                                skip_group_check=True)
                    rr = sbp.tile([128, CW], F32, tag="rr")
                    nc.scalar.activation(rr[:], pr[:], AF.Sigmoid)
                    rhb = sbp.tile([128, CW], BF16, tag="rhb")
                    nc.vector.tensor_mul(rhb[:], rr[:], hf[:])
                    nc.tensor.matmul(pc[:], identw[:],
                                     S[:, u * SC + GW:(u + 1) * SC],
                                     start=True, stop=False,
                                     skip_group_check=True)
                    for m in range(KB):
                        for k in range(KB):
                            nc.tensor.matmul(
                                pc[:, m * NSEQ:(m + 1) * NSEQ],
                                wchs[:, ts(k * KB + m, 128)],
                                rhb[:, k * NSEQ:(k + 1) * NSEQ],
                                start=False,
                                stop=(m == KB - 1 and k == KB - 1),
                                skip_group_check=True)
                    uu = sbp.tile([128, CW], F32, tag="uu")
                    nc.scalar.activation(uu[:], pu[:], AF.Sigmoid)
                    vv = sbp.tile([128, CW], F32, tag="vv")
                    nc.scalar.activation(vv[:], pu[:], AF.Sigmoid, scale=-1.0)
                    pp = sbp.tile([128, CW], F32, tag="pp")
                    nc.vector.tensor_mul(pp[:], uu[:], hf[:])
                    cc = sbp.tile([128, CW], F32, tag="cc")
                    nc.scalar.activation(cc[:], pc[:], AF.Tanh)
                    # state update, twice in parallel: f32 on Vector for the
                    # next step's r*h / u*h, bf16 on Pool for the matmuls.
                    m2 = sbp.tile([128, CW], F32, tag="m2")
                    nc.vector.tensor_mul(m2[:], vv[:], cc[:])
                    nc.vector.tensor_add(hf[:], pp[:], m2[:])
                    pm2 = sbp.tile([128, CW], F32, tag="pm2")
                    nc.gpsimd.tensor_mul(pm2[:], vv[:], cc[:])
                    nc.gpsimd.tensor_add(hist_dst, pp[:], pm2[:])

                def emit_logits(hist, iv, sub):
                    psl = pslp.tile([C, U * NSEQ], F32, tag="psl")
                    for k in range(KB):
                        nc.tensor.matmul(
                            psl[:], wfs[:, ts(k, C)],
                            hist[:, :, k * NSEQ:(k + 1) * NSEQ],
                            start=(k == 0), stop=(k == KB - 1))
                    ls = lstp.tile([C, U * NSEQ], F32, tag="ls")
                    nc.vector.tensor_copy(ls[:], psl[:])
                    nc.sync.dma_start(lview[iv, sub], ls[:])

                # prologue: load sub-chunk 0 into slot A
                nc.sync.dma_start(streamA[:], stream[0, :, :])
                with tc.For_i(0, NT, 1,
                              hint_engines=(mybir.EngineType.PE,)) as iv:
                    nc.sync.dma_start(streamB[:], sviewB[iv, 0])
                    for u in range(U):
                        hb = histB[:, U - 1, :] if u == 0 else histA[:, u - 1, :]
                        emit_step(u, hb, histA[:, u, :])
                    emit_logits(histA, iv, 0)
                    nc.sync.dma_start(streamA[:], sviewA2[iv, 0])
                    for u in range(U):
                        hb = histA[:, U - 1, :] if u == 0 else histB[:, u - 1, :]
                        emit_step(U + u, hb, histB[:, u, :])
                    emit_logits(histB, iv, 1)

    nc.compile()
    _NC_CACHE[key] = nc
    return nc


# ---------------- host-side helpers ----------------

def _reverse_sequence_np(x, lens):
    t = np.arange(x.shape[1])
    idx = np.where(t[None, :] < lens[:, None],
                   lens[:, None] - 1 - t[None, :], t[None, :])
    return np.take_along_axis(x, idx.reshape(idx.shape + (1,) * (x.ndim - 2)),
                              axis=1)


def _block_kxm(w, kb, mb):
    """[kb*128, mb*mw] -> [128, kb*mb*mw] with block (k,m) at cols (k*mb+m)*mw."""
    mw = w.shape[1] // mb
    return np.ascontiguousarray(
        w.reshape(kb, 128, mb, mw).transpose(1, 0, 2, 3).reshape(128, -1))


def make_core_inputs(core, inputs, T=T_FULL):
    """Build the per-core in_map from the full problem inputs."""
    x = np.asarray(inputs["x"], np.float32)
    lens = np.asarray(inputs["seq_lens"], np.int32)
    fwd = core < 4
    s0 = (core % 4) * NSEQ
    if fwd:
        xs = x[s0:s0 + NSEQ, :T]
        wg, bg, wc, bc = (inputs[k] for k in ("wg_f", "bg_f", "wc_f", "bc_f"))
        wf_half = np.asarray(inputs["wf"], np.float32)[0:H, :]
    else:
        x_rev = _reverse_sequence_np(x[:, :T], lens.clip(max=T))
        xs = x_rev[s0:s0 + NSEQ]
        wg, bg, wc, bc = (inputs[k] for k in ("wg_b", "bg_b", "wc_b", "bc_b"))
        wf_half = np.asarray(inputs["wf"], np.float32)[H:2 * H, :]
    wg = np.asarray(wg, np.float32)
    wc = np.asarray(wc, np.float32)
    clens = lens[s0:s0 + NSEQ].clip(max=T)
    maskrow = np.where(np.arange(T)[:, None] >= clens[None, :],
                       np.float32(MASK_BIG), np.float32(0.0)).astype(np.float32)
    mask_big = np.ascontiguousarray(
        np.broadcast_to(maskrow.reshape(1, T * NSEQ), (128, T * NSEQ)))
    bf16 = ml_dtypes.bfloat16
    return {
        "xT": np.ascontiguousarray(xs.transpose(2, 1, 0)).astype(bf16),
        "w1": np.asarray(inputs["w1"], np.float32).astype(bf16),
        "b1": np.asarray(inputs["b1"], np.float32),
        "w2": np.asarray(inputs["w2"], np.float32).astype(bf16),
        "b2": np.asarray(inputs["b2"], np.float32),
        "wgx": wg[0:H, :].astype(bf16),
        "bg": np.asarray(bg, np.float32),
        "wcx": wc[0:H, :].astype(bf16),
        "bc": np.asarray(bc, np.float32),
        "wgh": _block_kxm(wg[H:2 * H, :], KB, 2 * KB).astype(bf16),
        "wch": _block_kxm(wc[H:2 * H, :], KB, KB).astype(bf16),
        "wf": _block_kxm(wf_half, KB, 1).astype(bf16),
        "ident": np.eye(128, dtype=np.float32).astype(bf16),
        "maskb": mask_big,
    }


def kernel(**inputs):
    global LAST_RESULT
    nc = build_nc()
    in_maps = [make_core_inputs(core, inputs) for core in range(NCORES)]
    trace = bool(int(os.environ.get("GRU_TRACE", "0")))
    if trace:
        try:  # NTFF profiling under axon needs this hook; absent in some envs
            from antenv.axon_hooks import get_axon_ntff_profile_hook  # noqa: F401
        except ImportError:
            trace = False
    res = run_bass_kernel_spmd(nc, in_maps, core_ids=list(range(NCORES)),
                               trace=trace)
    LAST_RESULT = res

    x = np.asarray(inputs["x"], np.float32)
    lens = np.asarray(inputs["seq_lens"], np.int32)
    bf = np.asarray(inputs["bf"], np.float32)
    T = x.shape[1]
    pf = np.zeros((B, T, C), np.float32)
    pb_rev = np.zeros((B, T, C), np.float32)
    U = U_DEF
    for core in range(NCORES):
        s0 = (core % 4) * NSEQ
        lg = np.asarray(res.results[core]["logits"])  # [NSUB, C, U*NSEQ]
        part = (lg.reshape(T // U, C, U, NSEQ)
                .transpose(3, 0, 2, 1).reshape(NSEQ, T, C))
        if core < 4:
            pf[s0:s0 + NSEQ] = part
        else:
            pb_rev[s0:s0 + NSEQ] = part
    pb = _reverse_sequence_np(pb_rev, lens.clip(max=T))
    logits = pf + pb + bf[None, None, :]
    valid = np.arange(T)[None, :, None] < lens[:, None, None]
    logits = np.where(valid, logits, bf[None, None, :]).astype(np.float32)
    return logits


# revision 15
# speedup vs baseline: 1.3478x; 1.3478x over previous
"""Trainium2 Bass kernel for nn_CTCModel (bidirectional GRU CTC model).

Sharding: direction-split data parallel. Cores 0-3 run the forward GRU for
8 sequences each; cores 4-7 run the backward GRU (on host-reversed input)
for the same sequences. Each core:
  phase 1: bf16 MLP (relu(x@w1+b1) -> relu(@w2+b2)) and precompute of the
           x-dependent GRU terms gx = h2@wg_x+bg, cx = h2@wc_x+bc, with a
           +1e9 bias folded into the u-gate for t >= seq_len (freezes the
           recurrent state exactly, since u=sigmoid(1e9)=1). Stored to a
           DRAM stream in f32, one block per U-step sub-chunk.
  scan:    1000 sequential GRU steps, h kept feature-major [128, 4*8].
           Per step the Pool engine preloads gx/cx into PSUM two steps
           ahead; the gate/cand matmuls accumulate onto them (start=False)
           so sigmoid/tanh read PSUM directly with no vector adds on the
           critical path. r-gate matmuls are emitted before u-gate ones so
           sigmoid(r) overlaps the u-gate matmuls. The state update is
           computed twice in parallel: Vector writes f32 hf, Pool writes
           bf16 into a per-sub-chunk history tile consumed by the (batched,
           per-sub-chunk) output projection h @ wf_half.
Host glue: shard/reverse/transpose inputs, block weights, and combine
partial logits: logits = where(t < len, pf + pb + bf, bf).
"""
import os
import sys

sys.path.insert(0, "/opt/trn_rl_repo")

import numpy as np
import ml_dtypes

import concourse.bass as bass
import concourse.tile as tile
from concourse import bacc, mybir
from concourse.bass import ts
from concourse.bass_utils import run_bass_kernel_spmd

AF = mybir.ActivationFunctionType
F32 = mybir.dt.float32
BF16 = mybir.dt.bfloat16

B, T_FULL, F, H, C = 32, 1000, 161, 512, 62
NCORES = 8
NSEQ = 8  # sequences per core
U_DEF = 20  # steps per sub-chunk in the scan loop
MASK_BIG = 1.0e9
KB = H // 128  # 4 k-blocks of the hidden dim
GW = 2 * KB * NSEQ  # 64: gate cols per step (8 m-tiles x 8 seqs)
CW = KB * NSEQ  # 32: cand cols per step
SC = GW + CW  # 96: stream cols per step

LAST_RESULT = None  # BassKernelResults of the most recent run (for test.py)

_NC_CACHE = {}


def build_nc(T=T_FULL, U=U_DEF, ncores=NCORES):
    """Build + compile the per-core Bass program (same program on all cores)."""
    key = (T, U, ncores)
    if key in _NC_CACHE:
        return _NC_CACHE[key]
    assert T % (2 * U) == 0
    NT = T // (2 * U)
    NSUB = T // U
    N = T * NSEQ  # flattened (t, b) rows
    CH_T = (64 // U) * U if U <= 64 else U  # phase-1 chunk t-width
    assert CH_T % U == 0

    nc = bacc.Bacc("TRN2", target_bir_lowering=False, debug=False,
                   num_devices=ncores)

    # ---- DRAM I/O ----
    xT_d = nc.dram_tensor("xT", [F, T, NSEQ], BF16, kind="ExternalInput").ap()
    w1_d = nc.dram_tensor("w1", [F, H], BF16, kind="ExternalInput").ap()
    b1_d = nc.dram_tensor("b1", [H], F32, kind="ExternalInput").ap()
    w2_d = nc.dram_tensor("w2", [H, H], BF16, kind="ExternalInput").ap()
    b2_d = nc.dram_tensor("b2", [H], F32, kind="ExternalInput").ap()
    wgx_d = nc.dram_tensor("wgx", [H, 2 * H], BF16, kind="ExternalInput").ap()
    bg_d = nc.dram_tensor("bg", [2 * H], F32, kind="ExternalInput").ap()
    wcx_d = nc.dram_tensor("wcx", [H, H], BF16, kind="ExternalInput").ap()
    bc_d = nc.dram_tensor("bc", [H], F32, kind="ExternalInput").ap()
    wgh_d = nc.dram_tensor("wgh", [128, 2 * KB * KB * 128], BF16,
                           kind="ExternalInput").ap()  # [128, 4096]
    wch_d = nc.dram_tensor("wch", [128, KB * KB * 128], BF16,
                           kind="ExternalInput").ap()  # [128, 2048]
    wf_d = nc.dram_tensor("wf", [128, KB * C], BF16, kind="ExternalInput").ap()
    id_d = nc.dram_tensor("ident", [128, 128], BF16, kind="ExternalInput").ap()
    mask_d = nc.dram_tensor("maskb", [128, N], F32, kind="ExternalInput").ap()
    logits_d = nc.dram_tensor("logits", [NSUB, C, U * NSEQ], F32,
                              kind="ExternalOutput").ap()

    with tile.TileContext(nc) as tc:
        with tc.tile_pool(name="dram", bufs=1, space="DRAM") as dpool:
            # stream blocks: one [128, U*SC] block per sub-chunk; 2 pad
            # blocks for the software-pipeline prefetch overrun at the end.
            stream = dpool.tile([NSUB + 2, 128, U * SC], BF16)

            # ================= phase 1 =================
            with tc.tile_pool(name="p1w", bufs=1) as p1w, \
                 tc.tile_pool(name="p1work", bufs=2) as p1wk, \
                 tc.tile_pool(name="p1ps", bufs=2, space="PSUM") as p1ps:
                w1a = p1w.tile([128, H], BF16)
                nc.sync.dma_start(w1a[:], w1_d[0:128, :])
                w1b = p1w.tile([F - 128, H], BF16)
                nc.sync.dma_start(w1b[:], w1_d[128:F, :])
                w2t = p1w.tile([128, KB, H], BF16)
                nc.sync.dma_start(w2t[:], w2_d.rearrange("(k p) m -> p k m", p=128))
                wgxt = p1w.tile([128, KB, 2 * H], BF16)
                nc.sync.dma_start(wgxt[:], wgx_d.rearrange("(k p) m -> p k m", p=128))
                wcxt = p1w.tile([128, KB, H], BF16)
                nc.sync.dma_start(wcxt[:], wcx_d.rearrange("(k p) m -> p k m", p=128))
                b1t = p1w.tile([128, KB], F32)
                nc.sync.dma_start(b1t[:], b1_d.rearrange("(m p) -> p m", p=128))
                b2t = p1w.tile([128, KB], F32)
                nc.sync.dma_start(b2t[:], b2_d.rearrange("(m p) -> p m", p=128))
                bgt = p1w.tile([128, 2 * KB], F32)
                nc.sync.dma_start(bgt[:], bg_d.rearrange("(m p) -> p m", p=128))
                bct = p1w.tile([128, KB], F32)
                nc.sync.dma_start(bct[:], bc_d.rearrange("(m p) -> p m", p=128))

                n_chunks = (T + CH_T - 1) // CH_T
                for ci in range(n_chunks):
                    t0 = ci * CH_T
                    tw = min(CH_T, T - t0)
                    ns = tw // U  # sub-chunks in this chunk
                    cw = tw * NSEQ
                    c0 = t0 * NSEQ
                    stripe = p1wk.tile([128, CH_T // U, U, SC], BF16,
                                       tag="stripe")

                    xa = p1wk.tile([128, 512], BF16, tag="xa")
                    nc.sync.dma_start(
                        xa[:, 0:cw],
                        xT_d[0:128, t0:t0 + tw, :].rearrange("f t b -> f (t b)"))
                    xb = p1wk.tile([F - 128, 512], BF16, tag="xb")
                    nc.sync.dma_start(
                        xb[:, 0:cw],
                        xT_d[128:F, t0:t0 + tw, :].rearrange("f t b -> f (t b)"))
                    maskt = p1wk.tile([128, 512], F32, tag="maskt")
                    nc.sync.dma_start(maskt[:, 0:cw], mask_d[:, c0:c0 + cw])

                    # MLP layer 1: h1 = relu(w1.T @ x + b1)
                    h1t = p1wk.tile([128, KB, 512], BF16, tag="h1t")
                    for m in range(KB):
                        ps = p1ps.tile([128, 512], F32, tag="ps1")
                        nc.tensor.matmul(ps[:, 0:cw], w1a[:, ts(m, 128)],
                                         xa[:, 0:cw], start=True, stop=False)
                        nc.tensor.matmul(ps[:, 0:cw], w1b[:, ts(m, 128)],
                                         xb[:, 0:cw], start=False, stop=True)
                        nc.scalar.activation(h1t[:, m, 0:cw], ps[:, 0:cw],
                                             AF.Relu, bias=b1t[:, m:m + 1])
                    # MLP layer 2
                    h2t = p1wk.tile([128, KB, 512], BF16, tag="h2t")
                    for m in range(KB):
                        ps = p1ps.tile([128, 512], F32, tag="ps2")
                        for k in range(KB):
                            nc.tensor.matmul(ps[:, 0:cw],
                                             w2t[:, k, ts(m, 128)],
                                             h1t[:, k, 0:cw],
                                             start=(k == 0), stop=(k == KB - 1))
                        nc.scalar.activation(h2t[:, m, 0:cw], ps[:, 0:cw],
                                             AF.Relu, bias=b2t[:, m:m + 1])
                    # gate precompute gxb = h2 @ wg_x + bg (+ mask on u half)
                    for m in range(2 * KB):
                        ps = p1ps.tile([128, 512], F32, tag="psg")
                        for k in range(KB):
                            nc.tensor.matmul(ps[:, 0:cw],
                                             wgxt[:, k, ts(m, 128)],
                                             h2t[:, k, 0:cw],
                                             start=(k == 0), stop=(k == KB - 1))
                        gdst = stripe[:, 0:ns, :, m * NSEQ:(m + 1) * NSEQ]
                        gsrc = ps[:, 0:cw].rearrange("p (s t b) -> p s t b",
                                                     t=U, b=NSEQ)
                        if m < KB:
                            nc.scalar.activation(gdst, gsrc, AF.Identity,
                                                 bias=bgt[:, m:m + 1])
                        else:
                            nc.vector.scalar_tensor_tensor(
                                gdst, gsrc, bgt[:, m:m + 1],
                                maskt[:, 0:cw].rearrange(
                                    "p (s t b) -> p s t b", t=U, b=NSEQ),
                                mybir.AluOpType.add, mybir.AluOpType.add)
                    # cand precompute cxb = h2 @ wc_x + bc
                    for m in range(KB):
                        ps = p1ps.tile([128, 512], F32, tag="psc")
                        for k in range(KB):
                            nc.tensor.matmul(ps[:, 0:cw],
                                             wcxt[:, k, ts(m, 128)],
                                             h2t[:, k, 0:cw],
                                             start=(k == 0), stop=(k == KB - 1))
                        cdst = stripe[:, 0:ns, :,
                                      GW + m * NSEQ:GW + (m + 1) * NSEQ]
                        csrc = ps[:, 0:cw].rearrange("p (s t b) -> p s t b",
                                                     t=U, b=NSEQ)
                        nc.scalar.activation(cdst, csrc, AF.Identity,
                                             bias=bct[:, m:m + 1])
                    # store assembled sub-chunk blocks (contiguous per
                    # partition: U*SC elements -> 128 descriptors)
                    for s in range(ns):
                        nc.sync.dma_start(
                            stream[t0 // U + s, :, :],
                            stripe[:, s, :, :].rearrange("p t c -> p (t c)"))

            # ================= scan =================
            with tc.tile_pool(name="scw", bufs=1) as scw, \
                 tc.tile_pool(name="scstep", bufs=2) as sbp, \
                 tc.tile_pool(name="lstp", bufs=2) as lstp, \
                 tc.tile_pool(name="psw", bufs=1, space="PSUM") as pswp, \
                 tc.tile_pool(name="psl", bufs=2, space="PSUM") as pslp:
                wghs = scw.tile([128, 2 * KB * KB * 128], BF16)
                nc.sync.dma_start(wghs[:], wgh_d[:])
                wchs = scw.tile([128, KB * KB * 128], BF16)
                nc.sync.dma_start(wchs[:], wch_d[:])
                wfs = scw.tile([128, KB * C], BF16)
                nc.sync.dma_start(wfs[:], wf_d[:])
                identw = scw.tile([128, 128], BF16)
                nc.sync.dma_start(identw[:], id_d[:])
                hf = scw.tile([128, CW], F32)
                nc.vector.memset(hf[:], 0.0)
                histA = scw.tile([128, U, CW], BF16)
                histB = scw.tile([128, U, CW], BF16)
                nc.gpsimd.memset(histB[:], 0.0)
                streamA = scw.tile([128, U * SC], BF16)
                streamB = scw.tile([128, U * SC], BF16)
                histM = [histA, histB]
                sb = [streamA, streamB]

                NSUBB = 10  # sub-chunks per loop body
                assert NSUB % NSUBB == 0 and NSUBB % 2 == 0
                NTB = NSUB // NSUBB  # loop iterations
                NS2 = NSUBB * U  # steps per loop body
                # sviewN[iv, s] = stream block iv*NSUBB + s + 1: the block
                # prefetched at the start of sub-chunk s for sub-chunk s+1.
                sviewN = stream[1:1 + NTB * NSUBB].rearrange(
                    "(n s) f w -> n s f w", s=NSUBB)
                lview = logits_d.rearrange("(n s) c w -> n s c w", s=NSUBB)

                # persistent even/odd PSUM tiles for the per-step groups
                # (one full bank each; 6 + 2 logits banks = all 8)
                psrs = [pswp.tile([128, CW], F32, name=f"psr{i}")
                        for i in range(2)]
                psus = [pswp.tile([128, CW], F32, name=f"psu{i}")
                        for i in range(2)]
                pscs = [pswp.tile([128, CW], F32, name=f"psc{i}")
                        for i in range(2)]

                def emit_ids(j):
                    """Seed step j's three PSUM groups with the x-dependent
                    terms via identity matmuls (start=True resets the bank).
                    Emitted one step early so they fill PE gaps."""
                    jm = j % NS2
                    Sb = sb[(jm // U) % 2]
                    u = jm % U
                    pr, pu, pc = psrs[j % 2], psus[j % 2], pscs[j % 2]
                    nc.tensor.matmul(pr[:], identw[:],
                                     Sb[:, u * SC:u * SC + CW],
                                     start=True, stop=False,
                                     skip_group_check=True)
                    nc.tensor.matmul(pu[:], identw[:],
                                     Sb[:, u * SC + CW:u * SC + GW],
                                     start=True, stop=False,
                                     skip_group_check=True)
                    nc.tensor.matmul(pc[:], identw[:],
                                     Sb[:, u * SC + GW:(u + 1) * SC],
                                     start=True, stop=False,
                                     skip_group_check=True)

                def emit_step(j, hb_prev, hist_dst):
                    """One GRU step. hb_prev: [128, CW] bf16 AP of h_{t-1};
                    hist_dst: [128, CW] bf16 AP to write h_t into."""
                    pr, pu, pc = psrs[j % 2], psus[j % 2], pscs[j % 2]
                    # r-gate matmuls first so sigmoid(r) can start while the
                    # u-gate matmuls still run on PE.
                    for m in range(KB):
                        for k in range(KB):
                            nc.tensor.matmul(
                                pr[:, m * NSEQ:(m + 1) * NSEQ],
                                wghs[:, ts(k * 2 * KB + m, 128)],
                                hb_prev[:, k * NSEQ:(k + 1) * NSEQ],
                                start=False,
                                stop=(m == KB - 1 and k == KB - 1),
                                skip_group_check=True)
                    for m in range(KB):
                        for k in range(KB):
                            nc.tensor.matmul(
                                pu[:, m * NSEQ:(m + 1) * NSEQ],
                                wghs[:, ts(k * 2 * KB + KB + m, 128)],
                                hb_prev[:, k * NSEQ:(k + 1) * NSEQ],
                                start=False,
                                stop=(m == KB - 1 and k == KB - 1),
                                skip_group_check=True)
                    # seeds for step j+1 keep the PE busy through the
                    # sigmoid/mul gap so the cand matmuls start warm.
                    emit_ids(j + 1)
                    rr = sbp.tile([128, CW], F32, tag="rr")
                    nc.scalar.activation(rr[:], pr[:], AF.Sigmoid)
                    rhb = sbp.tile([128, CW], BF16, tag="rhb")
                    nc.vector.tensor_mul(rhb[:], rr[:], hf[:])
                    for m in range(KB):
                        for k in range(KB):
                            nc.tensor.matmul(
                                pc[:, m * NSEQ:(m + 1) * NSEQ],
                                wchs[:, ts(k * KB + m, 128)],
                                rhb[:, k * NSEQ:(k + 1) * NSEQ],
                                start=False,
                                stop=(m == KB - 1 and k == KB - 1),
                                skip_group_check=True)
                    uu = sbp.tile([128, CW], F32, tag="uu")
                    nc.scalar.activation(uu[:], pu[:], AF.Sigmoid)
                    vv = sbp.tile([128, CW], F32, tag="vv")
                    nc.scalar.activation(vv[:], pu[:], AF.Sigmoid, scale=-1.0)
                    pp = sbp.tile([128, CW], F32, tag="pp")
                    nc.vector.tensor_mul(pp[:], uu[:], hf[:])
                    cc = sbp.tile([128, CW], F32, tag="cc")
                    nc.scalar.activation(cc[:], pc[:], AF.Tanh)
                    # state update on Vector: bf16 history first (feeds the
                    # next step's matmuls), f32 state off the critical path.
                    m2 = sbp.tile([128, CW], F32, tag="m2")
                    nc.vector.tensor_mul(m2[:], vv[:], cc[:])
                    nc.vector.tensor_add(hist_dst, pp[:], m2[:])
                    nc.vector.tensor_add(hf[:], pp[:], m2[:])

                def emit_logits(hist, iv, sub):
                    psl = pslp.tile([C, U * NSEQ], F32, tag="psl")
                    for k in range(KB):
                        nc.tensor.matmul(
                            psl[:], wfs[:, ts(k, C)],
                            hist[:, :, k * NSEQ:(k + 1) * NSEQ],
                            start=(k == 0), stop=(k == KB - 1))
                    ls = lstp.tile([C, U * NSEQ], F32, tag="ls")
                    nc.vector.tensor_copy(ls[:], psl[:])
                    nc.sync.dma_start(lview[iv, sub], ls[:])

                # prologue: load sub-chunk 0 into slot A, seed step 0
                nc.sync.dma_start(streamA[:], stream[0, :, :])
                emit_ids(0)
                with tc.For_i(0, NTB, 1,
                              hint_engines=(mybir.EngineType.PE,)) as iv:
                    for s in range(NSUBB):
                        nc.sync.dma_start(sb[(s + 1) % 2][:], sviewN[iv, s])
                        for u in range(U):
                            hb = (histM[(s - 1) % 2][:, U - 1, :] if u == 0
                                  else histM[s % 2][:, u - 1, :])
                            emit_step(s * U + u, hb, histM[s % 2][:, u, :])
                            if u == 0 and s > 0:
                                emit_logits(histM[(s - 1) % 2], iv, s - 1)
                    emit_logits(histM[(NSUBB - 1) % 2], iv, NSUBB - 1)

    nc.compile()
    _NC_CACHE[key] = nc
    return nc


# ---------------- host-side helpers ----------------

def _reverse_sequence_np(x, lens):
    t = np.arange(x.shape[1])
    idx = np.where(t[None, :] < lens[:, None],
                   lens[:, None] - 1 - t[None, :], t[None, :])
    return np.take_along_axis(x, idx.reshape(idx.shape + (1,) * (x.ndim - 2)),
                              axis=1)


def _block_kxm(w, kb, mb):
    """[kb*128, mb*mw] -> [128, kb*mb*mw] with block (k,m) at cols (k*mb+m)*mw."""
    mw = w.shape[1] // mb
    return np.ascontiguousarray(
        w.reshape(kb, 128, mb, mw).transpose(1, 0, 2, 3).reshape(128, -1))


def make_core_inputs(core, inputs, T=T_FULL):
    """Build the per-core in_map from the full problem inputs."""
    x = np.asarray(inputs["x"], np.float32)
    lens = np.asarray(inputs["seq_lens"], np.int32)
    fwd = core < 4
    s0 = (core % 4) * NSEQ
    if fwd:
        xs = x[s0:s0 + NSEQ, :T]
        wg, bg, wc, bc = (inputs[k] for k in ("wg_f", "bg_f", "wc_f", "bc_f"))
        wf_half = np.asarray(inputs["wf"], np.float32)[0:H, :]
    else:
        x_rev = _reverse_sequence_np(x[:, :T], lens.clip(max=T))
        xs = x_rev[s0:s0 + NSEQ]
        wg, bg, wc, bc = (inputs[k] for k in ("wg_b", "bg_b", "wc_b", "bc_b"))
        wf_half = np.asarray(inputs["wf"], np.float32)[H:2 * H, :]
    wg = np.asarray(wg, np.float32)
    wc = np.asarray(wc, np.float32)
    clens = lens[s0:s0 + NSEQ].clip(max=T)
    maskrow = np.where(np.arange(T)[:, None] >= clens[None, :],
                       np.float32(MASK_BIG), np.float32(0.0)).astype(np.float32)
    mask_big = np.ascontiguousarray(
        np.broadcast_to(maskrow.reshape(1, T * NSEQ), (128, T * NSEQ)))
    bf16 = ml_dtypes.bfloat16
    return {
        "xT": np.ascontiguousarray(xs.transpose(2, 1, 0)).astype(bf16),
        "w1": np.asarray(inputs["w1"], np.float32).astype(bf16),
        "b1": np.asarray(inputs["b1"], np.float32),
        "w2": np.asarray(inputs["w2"], np.float32).astype(bf16),
        "b2": np.asarray(inputs["b2"], np.float32),
        "wgx": wg[0:H, :].astype(bf16),
        "bg": np.asarray(bg, np.float32),
        "wcx": wc[0:H, :].astype(bf16),
        "bc": np.asarray(bc, np.float32),
        "wgh": _block_kxm(wg[H:2 * H, :], KB, 2 * KB).astype(bf16),
        "wch": _block_kxm(wc[H:2 * H, :], KB, KB).astype(bf16),
        "wf": _block_kxm(wf_half, KB, 1).astype(bf16),
        "ident": np.eye(128, dtype=np.float32).astype(bf16),
        "maskb": mask_big,
    }


def kernel(**inputs):
    global LAST_RESULT
    nc = build_nc()
    in_maps = [make_core_inputs(core, inputs) for core in range(NCORES)]
    trace = bool(int(os.environ.get("GRU_TRACE", "0")))
    if trace:
        try:  # NTFF profiling under axon needs this hook; absent in some envs
            from antenv.axon_hooks import get_axon_ntff_profile_hook  # noqa: F401
        except ImportError:
            trace = False
    res = run_bass_kernel_spmd(nc, in_maps, core_ids=list(range(NCORES)),
                               trace=trace)
    LAST_RESULT = res

    x = np.asarray(inputs["x"], np.float32)
    lens = np.asarray(inputs["seq_lens"], np.int32)
    bf = np.asarray(inputs["bf"], np.float32)
    T = x.shape[1]
    pf = np.zeros((B, T, C), np.float32)
    pb_rev = np.zeros((B, T, C), np.float32)
    U = U_DEF
    for core in range(NCORES):
        s0 = (core % 4) * NSEQ
        lg = np.asarray(res.results[core]["logits"])  # [NSUB, C, U*NSEQ]
        part = (lg.reshape(T // U, C, U, NSEQ)
                .transpose(3, 0, 2, 1).reshape(NSEQ, T, C))
        if core < 4:
            pf[s0:s0 + NSEQ] = part
        else:
            pb_rev[s0:s0 + NSEQ] = part
    pb = _reverse_sequence_np(pb_rev, lens.clip(max=T))
    logits = pf + pb + bf[None, None, :]
    valid = np.arange(T)[None, :, None] < lens[:, None, None]
    logits = np.where(valid, logits, bf[None, None, :]).astype(np.float32)
    return logits
